# revision 61
# baseline (speedup 1.0000x reference)
"""Bass/Tile kernel builder for the bimamba encoder (nn_Encoder_3556232921377).

Per-core SPMD program (8 cores = 4 samples x 2 block-parities):
  state h, res : [128(D), L] in SBUF, d-major (time along free dim).
  per pair p in {0,1}:
    res = h + 2*res (p>0) ; hn = rmsnorm(res)*nw
    xz = in_proj(hn) -> xm (2 halves, padded, bf16), zt; sz = silu(zt)
    per dir in {0,1} (dir1 reads everything through reversed-time APs):
      xc = silu(conv(xm)+cb)   [conv = 4 diag-matmuls on PE, PSUM-accum]
      B_rep/C_rep = x_proj with 8x-replicated rows (bf16)
      dt = softplus(dt_proj(dbl_r)+dpb) ; dtx = dt*xc
      per group g (8 channels x 16 states on partitions):
        dt_rep via selector matmul ; dA = exp(svec * dt_rep)  [Act]
        dtx_rep via selector matmul ; evacuated by Pool engine (bf16)
        dbx = dtx_rep*B_rep ; h = tensor_tensor_scan(dA, dbx) ; w = h*C_rep
        y += sel2.T @ w  (PSUM, contracts the 16 states)
      y_total = xc*Dp + y ; gated with silu(z); accumulated over dirs
    out = out_proj(y_total) (bf16)
    exchange via pairwise ReduceScatter (bf16); h = out + flip(partner_out)
  final: rmsnorm(h + 2*res)*nfw -> out

Engine balance: Act does exps/silus/softplus + small evacs; Pool does the
dtx_rep evacuation + collectives; DVE does the scans and 2x-mode bf16 muls;
PE does all matmuls incl. the depthwise conv.
"""
import numpy as np
from contextlib import ExitStack

import concourse.bass as bass
import concourse.bacc as bacc
import concourse.tile as tile
import concourse.mybir as mybir

F32 = mybir.dt.float32
BF = mybir.dt.bfloat16
AF = mybir.ActivationFunctionType
OP = mybir.AluOpType

D = 128
DI = 256
N = 16
R = 8
KCONV = 4
NG = 32          # channel groups of 8
GPH = 16         # groups per DI-half
EPS = 1e-5


def flipf(ap):
    """Reverse the innermost free dim of an AP."""
    dims = [list(d) for d in ap.ap]
    s, c = dims[-1]
    return bass.AP(tensor=ap.tensor, offset=ap.offset + s * (c - 1),
                   ap=dims[:-1] + [[-s, c]])


def build(L=2048, Tc=512, sim_exchange=False, n_cores=8):
    assert L % Tc == 0
    NCH = L // Tc
    PAD = KCONV - 1
    nc = bacc.Bacc("TRN2", num_devices=n_cores, target_bir_lowering=False)

    # ---------------- DRAM I/O ----------------
    x_d = nc.dram_tensor("x", [D, L], F32, kind="ExternalInput")
    out_d = nc.dram_tensor("out", [D, L], F32, kind="ExternalOutput")
    in_lhsT_d = nc.dram_tensor("in_lhsT", [2, 4, 128, 128], BF, kind="ExternalInput")
    out_lhsT_d = nc.dram_tensor("out_lhsT", [2, 2, 128, 128], BF, kind="ExternalInput")
    convW_d = nc.dram_tensor("convW", [2, 2, 2, KCONV, 128, 128], BF, kind="ExternalInput")
    xpB_d = nc.dram_tensor("xpB_lhsT", [2, 2, 2, 128, 128], BF, kind="ExternalInput")
    xpC_d = nc.dram_tensor("xpC_lhsT", [2, 2, 2, 128, 128], BF, kind="ExternalInput")
    xpR_d = nc.dram_tensor("xpR_lhsT", [2, 2, 2, 128, 8], BF, kind="ExternalInput")
    dtw_d = nc.dram_tensor("dt_lhsT", [2, 2, 2, 8, 128], BF, kind="ExternalInput")
    svec_d = nc.dram_tensor("svec", [2, 2, 128, NG], F32, kind="ExternalInput")
    scal_d = nc.dram_tensor("scal", [2, 2, 2, 128, 8], F32, kind="ExternalInput")
    nw_d = nc.dram_tensor("nw", [2, 128, 1], F32, kind="ExternalInput")
    nfw_d = nc.dram_tensor("nfw", [128, 1], F32, kind="ExternalInput")
    isf_d = nc.dram_tensor("isf", [128, 1], F32, kind="ExternalInput")
    omisf_d = nc.dram_tensor("omisf", [128, 1], F32, kind="ExternalInput")
    sel2f_d = nc.dram_tensor("sel2f", [GPH, 128, 128], BF, kind="ExternalInput")
    onec_d = nc.dram_tensor("ones_col", [128, 1], F32, kind="ExternalInput")
    oner_d = nc.dram_tensor("ones_row", [1, 128], F32, kind="ExternalInput")

    if sim_exchange:
        other_d = [nc.dram_tensor(f"other{p}", [D, L], BF, kind="ExternalInput")
                   for p in range(2)]
        cc_in = cc_out = None
    else:
        # half-major layout so each half-L ReduceScatter input is contiguous
        cc_in = [nc.dram_tensor(f"cc_in{p}", [2, 2, D, L // 2], BF, kind="Internal")
                 for p in range(2)]
        cc_out = [nc.dram_tensor(f"cc_out{p}", [2, D, L // 2], BF, kind="Internal")
                  for p in range(2)]

    with tile.TileContext(nc) as tc:
        with ExitStack() as ctx:
            pers = ctx.enter_context(tc.tile_pool(name="pers", bufs=1))
            wts = ctx.enter_context(tc.tile_pool(name="wts", bufs=2))
            chk = ctx.enter_context(tc.tile_pool(name="chk", bufs=2))
            gA = ctx.enter_context(tc.tile_pool(name="gA", bufs=2))
            gB = ctx.enter_context(tc.tile_pool(name="gB", bufs=3))
            gC = ctx.enter_context(tc.tile_pool(name="gC", bufs=3))
            gR = ctx.enter_context(tc.tile_pool(name="gR", bufs=3))
            ps_s = ctx.enter_context(tc.tile_pool(name="ps_s", bufs=4, space="PSUM"))
            ps_y = ctx.enter_context(tc.tile_pool(name="ps_y", bufs=1, space="PSUM"))

            # ---- persistent state & consts ----
            h_t = pers.tile([D, L], F32, tag="h")
            res_t = pers.tile([D, L], F32, tag="res")
            nc.sync.dma_start(out=h_t, in_=x_d[:, :])
            nc.sync.dma_start(out=res_t, in_=x_d[:, :])

            cst = {}
            for nm, dt_, src in [("isf", F32, isf_d), ("omisf", F32, omisf_d),
                                 ("nfw", F32, nfw_d), ("onec", F32, onec_d),
                                 ("oner", None, oner_d)]:
                shp = list(src.shape)
                dt_ = dt_ or F32
                tt = pers.tile(shp, dt_, tag=nm, name=nm)
                nc.sync.dma_start(out=tt, in_=src[:, :] if len(shp) == 2 else src[:])
                cst[nm] = tt
            # scalars used as tensor_scalar operands must come via a DVE copy
            isf_c = pers.tile([128, 1], F32, tag="isfc")
            omisf_c = pers.tile([128, 1], F32, tag="omisfc")
            nfw_c = pers.tile([128, 1], F32, tag="nfwc")
            nc.vector.tensor_copy(isf_c, cst["isf"])
            nc.vector.tensor_copy(omisf_c, cst["omisf"])
            nc.vector.tensor_copy(nfw_c, cst["nfw"])
            sel2f_b = pers.tile([128, GPH, 128], BF, tag="sel2fb")
            nc.sync.dma_start(out=sel2f_b, in_=sel2f_d.transpose([1, 0, 2]))
            sel2f_t = [sel2f_b[:, gg, :] for gg in range(GPH)]
            eps_c = pers.tile([128, 1], F32, tag="epsc")
            nc.vector.memset(eps_c, EPS)

            xm = [pers.tile([128, L + 2 * PAD], BF, tag=f"xm{hh}", name=f"xm{hh}")
                  for hh in range(2)]
            sz = [pers.tile([128, L], BF, tag=f"sz{hh}", name=f"sz{hh}") for hh in range(2)]
            # per-direction double-buffered activations (dir1 front phase can
            # overlap dir0's group phase)
            xc = [[pers.tile([128, L], BF, tag=f"xc{d_}{hh}", name=f"xc{d_}{hh}")
                   for hh in range(2)] for d_ in range(2)]
            # dt and dtx packed in one tile so one DMA replicates both
            dtt = [[pers.tile([128, 2, L], BF, tag=f"dtt{d_}{hh}", name=f"dtt{d_}{hh}")
                    for hh in range(2)] for d_ in range(2)]
            dt = [[dtt[d_][hh][:, 0, :] for hh in range(2)] for d_ in range(2)]
            dtx = [[dtt[d_][hh][:, 1, :] for hh in range(2)] for d_ in range(2)]
            y_acc = [pers.tile([128, L], BF, tag=f"yacc{hh}", name=f"yacc{hh}")
                     for hh in range(2)]
            zt = y_acc  # zt lifetime (in_proj -> silu) precedes y_acc writes
            B_rep = [pers.tile([128, L], BF, tag=f"Brep{d_}", name=f"Brep{d_}")
                     for d_ in range(2)]
            C_rep = [pers.tile([128, L], BF, tag=f"Crep{d_}", name=f"Crep{d_}")
                     for d_ in range(2)]
            dblr = [pers.tile([8, L], BF, tag=f"dblr{d_}", name=f"dblr{d_}")
                    for d_ in range(2)]
            # alias exchange buffers onto xm (dead between conv and next in_proj)
            outblk = xm[0][:, 0:L]
            o_s = xm[1][:, 0:L]
            for hh in range(2):
                nc.vector.memset(xm[hh][:, 0:PAD], 0.0)
                nc.vector.memset(xm[hh][:, PAD + L:], 0.0)

            def rmsnorm_chunks(src_tile, w_ap, emit, out_dt=BF):
                """src [128, L]; for each chunk emit(c, normed_chunk_ap)."""
                for c in range(NCH):
                    sl = slice(c * Tc, (c + 1) * Tc)
                    rc = src_tile[:, sl]
                    sq = chk.tile([D, Tc], F32, tag="sq")
                    nc.scalar.activation(sq, rc, AF.Square)
                    ms = ps_s.tile([128, Tc], F32, tag="pss")
                    nc.tensor.matmul(ms[0:1, :], cst["onec"], sq, start=True, stop=True)
                    lg = chk.tile([1, Tc], F32, tag="lg")
                    nc.scalar.activation(lg, ms[0:1, :], AF.Ln, bias=eps_c[0:1, 0:1], scale=1.0 / D)
                    lgr = ps_s.tile([128, Tc], F32, tag="pss")
                    nc.tensor.matmul(lgr, cst["oner"], lg, start=True, stop=True)
                    rstd = chk.tile([D, Tc], F32, tag="rstd")
                    nc.scalar.activation(rstd, lgr, AF.Exp, scale=-0.5)
                    hn = chk.tile([D, Tc], out_dt, tag="hn")
                    nc.vector.scalar_tensor_tensor(hn, rc, w_ap, rstd,
                                                   op0=OP.mult, op1=OP.mult)
                    emit(c, hn)

            for p in range(2):
                if p > 0:
                    # res = h + 2*res  (in-place on res, chunked for pipelining)
                    for c in range(NCH):
                        sl = slice(c * Tc, (c + 1) * Tc)
                        nc.vector.scalar_tensor_tensor(res_t[:, sl], res_t[:, sl],
                                                       2.0, h_t[:, sl],
                                                       op0=OP.mult, op1=OP.add)

                # -------- pair weights (batched DMAs) --------
                inW_b = wts.tile([128, 4, 128], BF, tag="inW", name="inW")
                nc.sync.dma_start(out=inW_b, in_=in_lhsT_d[p].transpose([1, 0, 2]))
                inW = [inW_b[:, m, :] for m in range(4)]
                outW_b = wts.tile([128, 2, 128], BF, tag="outW", name="outW")
                nc.sync.dma_start(out=outW_b, in_=out_lhsT_d[p].transpose([1, 0, 2]))
                outW = [outW_b[:, m, :] for m in range(2)]
                nw_t = wts.tile([128, 1], F32, tag="nw")
                nc.sync.dma_start(out=nw_t, in_=nw_d[p])
                nw_c = wts.tile([128, 1], F32, tag="nwc")
                nc.vector.tensor_copy(nw_c, nw_t)

                # -------- rmsnorm + in_proj (acts: Square/Ln/Exp + Copy only) ----
                def emit_inproj(c, hn):
                    # evacuations on DVE: it is idle during the pair front and
                    # this keeps the Act engine free for the rmsnorm chain
                    sl = slice(c * Tc, (c + 1) * Tc)
                    for m in range(4):
                        xz = ps_s.tile([128, Tc], F32, tag="pss")
                        nc.tensor.matmul(xz, inW[m], hn, start=True, stop=True)
                        if m < 2:
                            nc.vector.tensor_copy(xm[m][:, PAD + c * Tc: PAD + (c + 1) * Tc],
                                                  xz)
                        else:
                            nc.vector.tensor_copy(zt[m - 2][:, sl], xz)

                rmsnorm_chunks(res_t, nw_c, emit_inproj)

                dctx = {}

                def front_steps(dr):
                    # ---- dir weights (batched DMAs) ----
                    convW_b = wts.tile([128, 2, KCONV, 128], BF, tag="convW", name="convW", bufs=2)
                    nc.sync.dma_start(out=convW_b, in_=convW_d[p, dr].transpose([2, 0, 1, 3]))
                    convW = [[convW_b[:, kh, k, :] for k in range(KCONV)] for kh in range(2)]
                    yield
                    xpB_b = wts.tile([128, 2, 128], BF, tag="xpB", name="xpB", bufs=2)
                    nc.sync.dma_start(out=xpB_b, in_=xpB_d[p, dr].transpose([1, 0, 2]))
                    xpB = [xpB_b[:, kh, :] for kh in range(2)]
                    xpC_b = wts.tile([128, 2, 128], BF, tag="xpC", name="xpC", bufs=2)
                    nc.sync.dma_start(out=xpC_b, in_=xpC_d[p, dr].transpose([1, 0, 2]))
                    xpC = [xpC_b[:, kh, :] for kh in range(2)]
                    xpR_b = wts.tile([128, 2, 8], BF, tag="xpR", name="xpR", bufs=2)
                    nc.sync.dma_start(out=xpR_b, in_=xpR_d[p, dr].transpose([1, 0, 2]))
                    xpR = [xpR_b[:, kh, :] for kh in range(2)]
                    dtw_b = wts.tile([8, 2, 128], BF, tag="dtw", name="dtw", bufs=2)
                    nc.sync.dma_start(out=dtw_b, in_=dtw_d[p, dr].transpose([1, 0, 2]))
                    dtw = [dtw_b[:, kh, :] for kh in range(2)]
                    svec_t = wts.tile([128, NG], F32, tag="svec")
                    nc.sync.dma_start(out=svec_t, in_=svec_d[p, dr])
                    scal_b = wts.tile([128, 2, 8], F32, tag="scal", name="scal", bufs=2)
                    nc.sync.dma_start(out=scal_b, in_=scal_d[p, dr].transpose([1, 0, 2]))
                    # route per-partition scalars through DVE (sync-wait slots)
                    scal_c = [wts.tile([128, 8], F32, tag=f"scalc{hh}", name=f"scalc{hh}", bufs=2) for hh in range(2)]
                    svec_c = wts.tile([128, NG], F32, tag="svecc")
                    for hh in range(2):
                        nc.vector.tensor_copy(scal_c[hh], scal_b[:, hh, :])
                    nc.vector.tensor_copy(svec_c, svec_t)

                    def win_c(hh, k, c):
                        """Chunk c of the k-tap window, [128, Tc]."""
                        if dr == 0:
                            a = k + c * Tc
                            return xm[hh][:, a: a + Tc]
                        # reversed window chunk: slice then flip
                        a = 2 * PAD - k + L - (c + 1) * Tc
                        return flipf(xm[hh][:, a: a + Tc])

                    # ---- conv on PE (4 diag matmuls, PSUM-accum) + silu evac ----
                    for hh in range(2):
                        for c in range(NCH):
                            sl = slice(c * Tc, (c + 1) * Tc)
                            cps = ps_s.tile([128, Tc], F32, tag="pss")
                            for k in range(KCONV):
                                nc.tensor.matmul(cps, convW[hh][k], win_c(hh, k, c),
                                                 start=(k == 0), stop=(k == KCONV - 1))
                            nc.scalar.activation(xc[dr][hh][:, sl], cps, AF.Silu,
                                                 bias=scal_c[hh][:, 4:5])
                            yield

                    # ---- x_proj: B_rep, C_rep (bf16), dbl_r (bf16) ----
                    for c in range(NCH):
                        sl = slice(c * Tc, (c + 1) * Tc)
                        for lhsTs, dest in ((xpB, B_rep[dr]), (xpC, C_rep[dr]),
                                            (xpR, dblr[dr])):
                            ps = ps_s.tile([128, Tc], F32, tag="pss")
                            m_sz = dest.shape[0]
                            nc.tensor.matmul(ps[0:m_sz, :], lhsTs[0], xc[dr][0][:, sl],
                                             start=True, stop=False)
                            nc.tensor.matmul(ps[0:m_sz, :], lhsTs[1], xc[dr][1][:, sl],
                                             start=False, stop=True)
                            # DVE has slack during both fronts
                            nc.vector.tensor_copy(dest[:, sl], ps[0:m_sz, :])
                        yield

                    # ---- dt = softplus(dt_proj + dpb), dtx = dt*xc ----
                    for hh in range(2):
                        for c in range(NCH):
                            sl = slice(c * Tc, (c + 1) * Tc)
                            ps = ps_s.tile([128, Tc], F32, tag="pss")
                            nc.tensor.matmul(ps, dtw[hh], dblr[dr][0:8, sl],
                                             start=True, stop=True)
                            # softplus(x+dpb) == ln(1 + exp(x+dpb)); dt_pre ~ -4.6
                            et = chk.tile([128, Tc], F32, tag="et")
                            nc.scalar.activation(et, ps, AF.Exp,
                                                 bias=scal_c[hh][:, 5:6])
                            nc.scalar.activation(dt[dr][hh][:, sl], et, AF.Ln, bias=1.0)
                            yield
                        nc.vector.tensor_mul(dtx[dr][hh], dt[dr][hh], xc[dr][hh])

                    if dr == 1:
                        # z-gates are first consumed at the end of dir0's
                        # groups; emit them late, in dir0's group-phase slack
                        for hh in range(2):
                            nc.scalar.activation(sz[hh], zt[hh], AF.Silu)
                        yield

                    dctx[dr] = dict(scal_c=scal_c, svec_c=svec_c)

                # ---- fronts + groups: dir0 front fully, then dir0 groups with
                #      dir1's front interleaved into its emission (fills the
                #      in-order Act/PE streams' slack), then dir1 groups ----
                def rep16(src, gg):
                    """AP replicating src rows [8gg:8gg+8] 16x across partitions.
                    Dest partition P reads src partition 8gg+P//16 (DMA only)."""
                    base = src[8 * gg: 8 * gg + 8]
                    dims = [list(d) for d in base.ap]
                    return bass.AP(tensor=base.tensor, offset=base.offset,
                                   ap=[dims[0], [0, GPH]] + dims[1:])

                for step in front_steps(0):
                    pass
                f1 = front_steps(1)

                for dr in range(2):
                    if dr == 1:
                        for step in f1:  # drain any leftover dir1 front work
                            pass
                    scal_c = dctx[dr]["scal_c"]
                    svec_c = dctx[dr]["svec_c"]
                    # groups are software-pipelined: y matmuls lag one group
                    # so PE's in-order stream never stalls on the scan
                    for hh in range(2):
                        y_ps = [ps_y.tile([128, Tc], F32, tag="psy", name=f"y_ps{c}",
                                          bufs=NCH) for c in range(NCH)]
                        pend = None  # (h_s, gg) waiting for its w-mul + y matmuls

                        def emit_wy(h_p, g_p, last=False):
                            w_s = gC.tile([128, L], BF, tag="gC")
                            nc.gpsimd.tensor_mul(w_s, h_p, C_rep[dr])
                            for c in range(NCH):
                                sl = slice(c * Tc, (c + 1) * Tc)
                                nc.tensor.matmul(y_ps[c], sel2f_t[g_p], w_s[:, sl],
                                                 start=(g_p == 0), stop=last)

                        for gg in range(GPH):
                            g = hh * GPH + gg
                            if dr == 0:
                                next(f1, None)  # interleave dir1 front emission
                            # replicate dt+dtx rows across the state lattice in
                            # ONE SBUF->SBUF DMA (no PE, no PSUM)
                            rep = gR.tile([128, 2, L], BF, tag="gR")
                            nc.sync.dma_start(out=rep,
                                              in_=rep16(dtt[dr][hh], gg))
                            dA_t = gA.tile([128, L], BF, tag="gA")
                            nc.scalar.activation(dA_t, rep[:, 0, :], AF.Exp,
                                                 scale=svec_c[:, g:g + 1])
                            dbx = gC.tile([128, L], BF, tag="gC")
                            if gg % 2 == 0:
                                nc.vector.tensor_mul(dbx, rep[:, 1, :], B_rep[dr])
                            else:
                                nc.gpsimd.tensor_mul(dbx, rep[:, 1, :], B_rep[dr])
                            if pend is not None:
                                emit_wy(*pend)
                            h_s = gB.tile([128, L], BF, tag="gB")
                            nc.vector.tensor_tensor_scan(h_s, dA_t, dbx, 0.0,
                                                         op0=OP.mult, op1=OP.add)
                            pend = (h_s, gg)
                        emit_wy(*pend, last=True)
                        # ---- evacuate y for this half ----
                        t1 = gB.tile([128, L], BF, tag="gB")
                        for c in range(NCH):
                            sl = slice(c * Tc, (c + 1) * Tc)
                            nc.vector.scalar_tensor_tensor(t1[:, sl], xc[dr][hh][:, sl],
                                                           scal_c[hh][:, 6:7],
                                                           y_ps[c],
                                                           op0=OP.mult, op1=OP.add)
                        if dr == 0:
                            nc.vector.tensor_mul(y_acc[hh], t1, sz[hh])
                        else:
                            t2 = gC.tile([128, L], BF, tag="gC")
                            nc.vector.tensor_mul(t2, t1, flipf(sz[hh]))
                            nc.vector.tensor_tensor(y_acc[hh], y_acc[hh], flipf(t2),
                                                    op=OP.add)

                # ---- out_proj + exchange, split in two L-halves.
                # The BACK half of outblk is what the partner needs first
                # (time-flip), so ship it first: its ReduceScatter then
                # overlaps the front half's compute, and h/res/rmsnorm chunks
                # 0..1 can start while the second collective is in flight.
                if sim_exchange:
                    for c in range(NCH):
                        sl = slice(c * Tc, (c + 1) * Tc)
                        ps = ps_s.tile([128, Tc], F32, tag="pss")
                        nc.tensor.matmul(ps, outW[0], y_acc[0][:, sl], start=True, stop=False)
                        nc.tensor.matmul(ps, outW[1], y_acc[1][:, sl], start=False, stop=True)
                        nc.scalar.activation(outblk[:, sl], ps, AF.Copy)
                    nc.sync.dma_start(out=o_s, in_=other_d[p][:, :])
                else:
                    for half in (1, 0):
                        for c2 in range(2):
                            c = 2 * half + c2
                            sl = slice(c * Tc, (c + 1) * Tc)
                            sl2 = slice(c2 * Tc, (c2 + 1) * Tc)
                            ps = ps_s.tile([128, Tc], F32, tag="pss")
                            nc.tensor.matmul(ps, outW[0], y_acc[0][:, sl],
                                             start=True, stop=False)
                            nc.tensor.matmul(ps, outW[1], y_acc[1][:, sl],
                                             start=False, stop=True)
                            nc.scalar.activation(outblk[:, sl], ps, AF.Copy)
                            s01 = chk.tile([128, 2, Tc], BF, tag="s01")
                            nc.vector.tensor_scalar_mul(s01[:, 0, :], outblk[:, sl],
                                                        omisf_c[:, 0:1])
                            nc.vector.tensor_scalar_mul(s01[:, 1, :], outblk[:, sl],
                                                        isf_c[:, 0:1])
                            nc.sync.dma_start(
                                out=cc_in[p][half][:, :, sl2].transpose([1, 0, 2]),
                                in_=s01)
                        hsl = slice(half * (L // 2), (half + 1) * (L // 2))
                        nc.gpsimd.collective_compute(
                            "ReduceScatter", OP.add,
                            replica_groups=[[0, 4], [1, 5], [2, 6], [3, 7]],
                            ins=[cc_in[p][half]], outs=[cc_out[p][half]])
                        nc.sync.dma_start(out=o_s[:, hsl], in_=cc_out[p][half])
                # h = outblk + flip(other), chunked (chunk c uses o_s chunk 3-c)
                for c in range(NCH):
                    sl = slice(c * Tc, (c + 1) * Tc)
                    osl = slice(L - (c + 1) * Tc, L - c * Tc)
                    nc.vector.tensor_tensor(h_t[:, sl], outblk[:, sl],
                                            flipf(o_s[:, osl]), op=OP.add)

            # -------- final: rmsnorm(h + 2*res), in-place on h --------
            for c in range(NCH):
                sl = slice(c * Tc, (c + 1) * Tc)
                nc.vector.scalar_tensor_tensor(h_t[:, sl], res_t[:, sl], 2.0,
                                               h_t[:, sl], op0=OP.mult, op1=OP.add)

            def emit_out(c, hn):
                sl = slice(c * Tc, (c + 1) * Tc)
                nc.sync.dma_start(out=out_d[:, sl], in_=hn)

            rmsnorm_chunks(h_t, nfw_c[:, 0:1], emit_out, out_dt=F32)

    nc.compile()
    return nc


# ---------------- host-side input prep ----------------

def make_core_inputs(x, w, L=2048, n_cores=8):
    """x [B, L, D] f32; w = weights dict (numpy). Returns list of per-core dicts."""
    B = x.shape[0]
    maps = []
    for c in range(n_cores):
        s, par = c % B, c // B
        xT = np.ascontiguousarray(x[s].T.astype(np.float32))       # [D, L]
        if par == 1:
            xT = np.ascontiguousarray(xT[:, ::-1])
        in_lhsT = np.zeros((2, 4, 128, 128), np.float32)
        out_lhsT = np.zeros((2, 2, 128, 128), np.float32)
        convW = np.zeros((2, 2, 2, KCONV, 128, 128), np.float32)
        xpB = np.zeros((2, 2, 2, 128, 128), np.float32)
        xpC = np.zeros((2, 2, 2, 128, 128), np.float32)
        xpR = np.zeros((2, 2, 2, 128, 8), np.float32)
        dtw = np.zeros((2, 2, 2, 8, 128), np.float32)
        svec = np.zeros((2, 2, 128, NG), np.float32)
        scal = np.zeros((2, 2, 2, 128, 8), np.float32)
        nw = np.zeros((2, 128, 1), np.float32)
        rng = np.arange(128)
        for p in range(2):
            bi = 2 * p + par
            ilT = w["in_proj_w"][bi].T                              # [128, 512]
            for m in range(4):
                in_lhsT[p, m] = ilT[:, m * 128:(m + 1) * 128]
            olT = w["out_proj_w"][bi].T                             # [256, 128]
            for kh in range(2):
                out_lhsT[p, kh] = olT[kh * 128:(kh + 1) * 128]
            for dr in range(2):
                for dh in range(2):
                    dsl = slice(dh * 128, (dh + 1) * 128)
                    for k in range(KCONV):
                        convW[p, dr, dh, k, rng, rng] = w["conv_w"][bi, dr][dsl, k]
                xpw = w["x_proj_w"][bi, dr]                         # [40, 256]
                # lattice layout: partition p -> (ch = 8g + p//16, st = p%16)
                BlT = np.tile(xpw[R:R + N], (8, 1)).T               # [256, 128]
                ClT = np.tile(xpw[R + N:], (8, 1)).T
                RlT = xpw[:R].T                                     # [256, 8]
                for kh in range(2):
                    xpB[p, dr, kh] = BlT[kh * 128:(kh + 1) * 128]
                    xpC[p, dr, kh] = ClT[kh * 128:(kh + 1) * 128]
                    xpR[p, dr, kh] = RlT[kh * 128:(kh + 1) * 128]
                dpw = w["dt_proj_w"][bi, dr]                        # [256, 8]
                for dh in range(2):
                    dtw[p, dr, dh] = dpw[dh * 128:(dh + 1) * 128].T
                A = -np.exp(w["A_log"][bi, dr])                     # [256, 16]
                pp = np.arange(128)
                for g in range(NG):
                    svec[p, dr, :, g] = A[8 * g + pp // 16, pp % 16]
                for dh in range(2):
                    dsl = slice(dh * 128, (dh + 1) * 128)
                    scal[p, dr, dh, :, 4] = w["conv_b"][bi, dr][dsl]
                    scal[p, dr, dh, :, 5] = w["dt_proj_b"][bi, dr][dsl]
                    scal[p, dr, dh, :, 6] = w["D_skip"][bi, dr][dsl]
            nw[p, :, 0] = w["norm_w"][bi]
        # y contraction: out channel m collects the 16 states of lattice rows
        # p = 16*(m-8g)..16*(m-8g)+15  ->  sel2f[g, p, m] = (m == 8g + p//16)
        sel2f = np.zeros((GPH, 128, 128), np.float32)
        for gg in range(GPH):
            pp = np.arange(128)
            sel2f[gg, pp, 8 * gg + pp // 16] = 1.0
        f = 1.0 if par == 0 else 0.0
        maps.append(dict(
            x=xT,
            in_lhsT=to_bf16(in_lhsT), out_lhsT=to_bf16(out_lhsT),
            convW=to_bf16(convW),
            xpB_lhsT=to_bf16(xpB), xpC_lhsT=to_bf16(xpC), xpR_lhsT=to_bf16(xpR),
            dt_lhsT=to_bf16(dtw), svec=svec, scal=scal, nw=nw,
            nfw=w["norm_f_w"].reshape(128, 1).astype(np.float32),
            isf=np.full((128, 1), f, np.float32),
            omisf=np.full((128, 1), 1.0 - f, np.float32),
            sel2f=to_bf16(sel2f),
            ones_col=np.ones((128, 1), np.float32),
            ones_row=np.ones((1, 128), np.float32),
        ))
    return maps


def to_bf16(a):
    import ml_dtypes
    return a.astype(ml_dtypes.bfloat16)


# ======================= harness entry point =======================
import os as _os

_NC_CACHE = {}
LAST_EXEC_TIME_NS = None
LAST_RESULT = None


def kernel(**inputs):
    """Full-input entry: x [B, L, D] f32 + weights; returns [B, L, D] f32."""
    global LAST_EXEC_TIME_NS, LAST_RESULT
    from concourse import bass_utils
    x = np.asarray(inputs["x"], dtype=np.float32)
    w = {k: np.asarray(v) for k, v in inputs.items() if k != "x"}
    B, L, _ = x.shape
    key = (L,)
    if key not in _NC_CACHE:
        _NC_CACHE[key] = build(L=L, Tc=512, sim_exchange=False)
    nc = _NC_CACHE[key]
    maps = make_core_inputs(x, w, L=L)
    trace = _os.environ.get("KERNEL_TRACE", "0") != "0"
    r = bass_utils.run_bass_kernel_spmd(nc, maps, core_ids=list(range(8)),
                                        trace=trace)
    LAST_EXEC_TIME_NS = r.exec_time_ns
    LAST_RESULT = r
    out = np.stack([np.asarray(r.results[s]["out"]).T for s in range(B)], axis=0)
    return out.astype(np.float32)


def bench(inputs, iters=20, n_cores=8):
    """Time the sharded PJRT executable with device-resident inputs.
    Returns (min_ns, med_ns, outputs_list)."""
    import time
    import jax
    from jax.sharding import Mesh, PartitionSpec, NamedSharding
    from jax.experimental.shard_map import shard_map
    from concourse import bass2jax

    x = np.asarray(inputs["x"], dtype=np.float32)
    w = {k: np.asarray(v) for k, v in inputs.items() if k != "x"}
    B, L, _ = x.shape
    key = (L,)
    if key not in _NC_CACHE:
        _NC_CACHE[key] = build(L=L, Tc=512, sim_exchange=False)
    nc = _NC_CACHE[key]
    maps = make_core_inputs(x, w, L=L)

    bass2jax.install_neuronx_cc_hook()
    partition_name = nc.partition_id_tensor.name if nc.partition_id_tensor else None
    in_names, out_names, out_avals, zero_outs = [], [], [], []
    for alloc in nc.m.functions[0].allocations:
        if not isinstance(alloc, mybir.MemoryLocationSet):
            continue
        name = alloc.memorylocations[0].name
        if alloc.kind == "ExternalInput":
            if name != partition_name:
                in_names.append(name)
        elif alloc.kind == "ExternalOutput":
            shape = tuple(alloc.tensor_shape)
            dtyp = mybir.dt.np(alloc.dtype)
            out_names.append(name)
            out_avals.append(jax.core.ShapedArray(shape, dtyp))
            zero_outs.append(np.zeros(shape, dtyp))
    n_params = len(in_names)
    n_outs = len(out_avals)
    all_in_names = list(in_names) + list(out_names)
    if partition_name is not None:
        all_in_names.append(partition_name)
    donate = tuple(range(n_params, n_params + n_outs))

    def _body(*args):
        operands = list(args)
        if partition_name is not None:
            operands.append(bass2jax.partition_id_tensor())
        outs = bass2jax._bass_exec_p.bind(
            *operands,
            out_avals=tuple(out_avals),
            in_names=tuple(all_in_names),
            out_names=tuple(out_names),
            lowering_input_output_aliases=(),
            sim_require_finite=True,
            sim_require_nnan=True,
            nc=nc,
        )
        return tuple(outs)

    devices = jax.devices()[:n_cores]
    mesh = Mesh(np.asarray(devices), ("core",))
    in_specs = (PartitionSpec("core"),) * (n_params + n_outs)
    out_specs = (PartitionSpec("core"),) * n_outs
    sharded = jax.jit(
        shard_map(_body, mesh=mesh, in_specs=in_specs, out_specs=out_specs,
                  check_rep=False),
        donate_argnums=donate, keep_unused=True)
    sh = NamedSharding(mesh, PartitionSpec("core"))
    concat_in = [
        jax.device_put(np.concatenate([np.asarray(maps[c][nm]) for c in range(n_cores)],
                                      axis=0), sh)
        for nm in in_names
    ]
    concat_zeros_np = [np.zeros((n_cores * z.shape[0], *z.shape[1:]), z.dtype)
                       for z in zero_outs]
    times = []
    outs = None
    for it in range(iters):
        zs = [jax.device_put(z, sh) for z in concat_zeros_np]
        for a in zs:
            a.block_until_ready()
        t0 = time.perf_counter()
        outs = sharded(*concat_in, *zs)
        for o in outs:
            o.block_until_ready()
        times.append((time.perf_counter() - t0) * 1e9)
    times.sort()
    res = [np.asarray(o) for o in outs]
    return int(times[0]), int(times[len(times) // 2]), (out_names, res)
